# revision 31
# baseline (speedup 1.0000x reference)
"""DeltaNet attention TRN2 kernel (nn_DeltaNetAttention_5299989643476).

Strategy: data-parallel over batch (8 batches -> 8 NeuronCores). The
cross-batch cumulative_state scan is tiny ([H, Dh]) and is computed on the
host via an algebraic shortcut (mean over (b,l) of kv == Ksum . V
contraction), then passed to every core as a small constant tensor, so the
device program needs no collectives.

On-device, everything runs in a "transposed" layout (features on
partitions, sequence on the free dim):
  - QT/KT/VT projections: weight-stationary bf16 matmuls, fp32 PSUM accum
  - per head: kvT matmul; q-mod via tensor_scalar with per-partition cs;
    phi(x)=elu(x)+1 = relu(x)+exp(min(x,0)); causal linear attention as a
    masked A=pq@pk^T matmul; den via an all-ones stationary matmul (which
    also replicates den across partitions for the division broadcast);
    num needs V back in sequence-major layout -> PE transpose
  - output projection + residual + LayerNorm (bn_stats/bn_aggr)
"""

import numpy as np
import ml_dtypes

import concourse.bass as bass
import concourse.mybir as mybir
import concourse.tile as tile
from concourse import bacc
from concourse.bass_utils import run_bass_kernel_spmd
from concourse.masks import make_identity

B, L, D, H = 8, 256, 2048, 8
DH = D // H            # 256
NB = D // 128          # 16 feature blocks of 128
LB = L // 128          # 2 sequence blocks of 128
EPS = 1e-5

F32 = mybir.dt.float32
BF16 = mybir.dt.bfloat16
AF = mybir.ActivationFunctionType
OP = mybir.AluOpType

_cache = {}


def _build(alpha: float, plain_ln: bool = False):
    nc = bacc.Bacc(
        "TRN2",
        target_bir_lowering=False,
        debug=False,
        enable_asserts=False,
        num_devices=B,
    )

    qT_d = nc.dram_tensor("qT", [D, L], BF16, kind="ExternalInput")
    kT_d = nc.dram_tensor("kT", [D, L], BF16, kind="ExternalInput")
    vT_d = nc.dram_tensor("vT", [D, L], BF16, kind="ExternalInput")
    qres_d = nc.dram_tensor("qres", [L, D], F32, kind="ExternalInput")
    wqT_d = nc.dram_tensor("wqT", [D, D], BF16, kind="ExternalInput")
    wkT_d = nc.dram_tensor("wkT", [D, D], BF16, kind="ExternalInput")
    wvT_d = nc.dram_tensor("wvT", [D, D], BF16, kind="ExternalInput")
    woT_d = nc.dram_tensor("woT", [D, D], BF16, kind="ExternalInput")
    csp_d = nc.dram_tensor("csp", [128, H * 2], F32, kind="ExternalInput")
    maskT_d = nc.dram_tensor("maskT", [L, L], BF16, kind="ExternalInput")
    lng_d = nc.dram_tensor("lng", [D], F32, kind="ExternalInput")
    lnb_d = nc.dram_tensor("lnb", [D], F32, kind="ExternalInput")
    out_d = nc.dram_tensor("out", [L, D], F32, kind="ExternalOutput")

    with tile.TileContext(nc) as tc:
        _body(
            tc, alpha,
            qT_d, kT_d, vT_d, qres_d,
            wqT_d, wkT_d, wvT_d, woT_d,
            csp_d, maskT_d, lng_d, lnb_d, out_d,
            plain_ln,
        )
    nc.compile()
    return nc


def _body(tc, alpha, qT_d, kT_d, vT_d, qres_d, wqT_d, wkT_d, wvT_d, woT_d,
          csp_d, maskT_d, lng_d, lnb_d, out_d, plain_ln):
    nc = tc.nc

    with (
        tc.tile_pool(name="singles", bufs=1) as singles,
        tc.tile_pool(name="wpool", bufs=4) as wpool,
        tc.tile_pool(name="big", bufs=1) as big,
        tc.tile_pool(name="hgrp", bufs=2) as hgrp,
        tc.tile_pool(name="small", bufs=3) as small,
        tc.tile_pool(name="proj_ps", bufs=2, space="PSUM") as proj_ps,
        tc.tile_pool(name="kv_ps", bufs=2, space="PSUM") as kv_ps,
        tc.tile_pool(name="an_ps", bufs=2, space="PSUM") as an_ps,
        tc.tile_pool(name="dv_ps", bufs=2, space="PSUM") as dv_ps,
    ):
        # ---- projections: XT[i, l] = sum_j WT[j, i] * xT[j, l] ----
        # K first (pk depends only on K), then V (kv + transposes), then Q.
        # Inputs stream on the gpsimd queue, weights on the sync queue, so
        # their issue costs overlap. The K input DMA goes first on gpsimd.
        xT_in = {}
        for name, dram in (("k", kT_d), ("v", vT_d), ("q", qT_d)):
            t = big.tile([128, NB, L], BF16, tag=f"{name}T_in", name=f"{name}T_in")
            xT_in[name] = (t, dram)

        def load_xT(name):
            t, dram = xT_in[name]
            r = dram.rearrange("(n p) l -> p n l", p=128)
            # halves so the first j-blocks unblock matmuls sooner
            nc.gpsimd.dma_start(out=t[:, 0:8, :], in_=r[:, 0:8, :])
            nc.gpsimd.dma_start(out=t[:, 8:16, :], in_=r[:, 8:16, :])

        load_xT("k")

        # constants after the K input on the gpsimd queue
        ident = singles.tile([128, 128], BF16)
        make_identity(nc, ident)
        ones_t = singles.tile([128, 128], BF16)
        nc.vector.memset(ones_t, 1.0)
        eps_t = singles.tile([128, 1], F32)
        nc.vector.memset(eps_t, EPS)
        csp_t = singles.tile([128, H * 2], F32)
        nc.gpsimd.dma_start(out=csp_t, in_=csp_d.ap())

        w_rs = {
            "k": wkT_d.rearrange("(n p) i -> p n i", p=128),
            "v": wvT_d.rearrange("(n p) i -> p n i", p=128),
            "q": wqT_d.rearrange("(n p) i -> p n i", p=128),
            "o": woT_d.rearrange("(n p) i -> p n i", p=128),
        }
        succ = {"k": "v", "v": "q", "q": "o"}
        prefetched = {}

        def panel_dma(name, iq, tag, halved=False):
            w_t = wpool.tile([128, NB, 256], BF16, tag=tag, name=f"w_{name}{iq}",
                             bufs=(2 if tag == "wf" else None))
            wsl = slice(iq * 256, (iq + 1) * 256)
            w_r = w_rs[name]
            if halved:
                nc.sync.dma_start(out=w_t[:, 0:8, :], in_=w_r[:, 0:8, wsl])
                nc.sync.dma_start(out=w_t[:, 8:16, :], in_=w_r[:, 8:16, wsl])
            else:
                nc.sync.dma_start(out=w_t, in_=w_r[:, :, wsl])
            return w_t

        projs = {}
        for name in ("k", "v", "q"):
            out_t = big.tile([128, NB, L], BF16, tag=f"{name}proj",
                             name=f"{name}proj")
            x_t = xT_in[name][0]
            for iq in range(8):  # i-quarter: 2 output feature blocks
                w_t = prefetched.pop((name, iq), None)
                if w_t is None:
                    w_t = panel_dma(name, iq, "w", halved=(name == "k" and iq < 2))
                if iq == 0 and name != "k":
                    load_xT(name)  # next activation streams during this proj
                if iq == 2:
                    # next matmul stream's first panel, in dedicated slots, so
                    # the projection boundary has no weight-arrival gap
                    nxt = succ[name]
                    prefetched[(nxt, 0)] = panel_dma(nxt, 0, "wf")
                ps = proj_ps.tile([128, 2, L], F32, tag="proj")
                for ib in range(2):
                    for j in range(NB):
                        nc.tensor.matmul(
                            ps[:, ib, :],
                            w_t[:, j, ib * 128:(ib + 1) * 128],
                            x_t[:, j, :],
                            start=(j == 0),
                            stop=(j == NB - 1),
                        )
                for ib in range(2):
                    nc.vector.tensor_copy(out_t[:, iq * 2 + ib, :], ps[:, ib, :])
            projs[name] = out_t
        KT_t, VT_t, QT_t = projs["k"], projs["v"], projs["q"]

        maskT_t = singles.tile([128, LB, L], BF16)
        nc.gpsimd.dma_start(out=maskT_t,
                            in_=maskT_d.rearrange("(a p) l -> p a l", p=128))
        qres_t = []
        for lb in range(LB):
            t = big.tile([128, D], F32, tag=f"qres{lb}", name=f"qres{lb}")
            nc.gpsimd.dma_start(out=t, in_=qres_d.ap()[lb * 128:(lb + 1) * 128, :])
            qres_t.append(t)
        lng_t = lnb_t = None
        if not plain_ln:
            lng_t = singles.tile([128, D], F32)
            nc.gpsimd.dma_start(out=lng_t,
                                in_=lng_d.ap().partition_broadcast(128))
            lnb_t = singles.tile([128, D], F32)
            nc.gpsimd.dma_start(out=lnb_t,
                                in_=lnb_d.ap().partition_broadcast(128))

        # ---- pk = phi(KT) over all heads at once ----
        pk_t = big.tile([128, NB, L], BF16, tag="pk")
        ek_t = big.tile([128, NB, L], BF16, tag="ek")
        nc.vector.tensor_scalar_min(ek_t, KT_t, 0.0)
        nc.scalar.activation(ek_t, ek_t, AF.Exp)
        nc.vector.tensor_scalar_max(pk_t, KT_t, 0.0)
        nc.vector.tensor_add(pk_t, pk_t, ek_t)

        # ---- per-head-group (2 heads): kv + V-transpose + q-mod + phi(q) ----
        # V-transposes ride along per group so PE has filler work while the
        # group's phi chain runs on DVE/ACT.
        V_t = big.tile([128, LB, D], BF16, tag="V")
        pq_t = big.tile([128, NB, L], BF16, tag="pq")
        for g in range(4):  # groups of 2 heads
            kvm = hgrp.tile([128, 4, L], BF16, tag="kvm")
            for hh in range(2):
                h = 2 * g + hh
                n0 = 2 * h
                ps = kv_ps.tile([128, 2, L], F32, tag="kv")
                for mb in range(2):
                    for db in range(2):
                        nc.tensor.matmul(
                            ps[:, mb, :],
                            VT_t[:, n0 + db, mb * 128:(mb + 1) * 128],
                            KT_t[:, n0 + db, :],
                            start=(db == 0),
                            stop=(db == 1),
                        )
                for mb in range(2):
                    # kvm = alpha*kv + (1-alpha)*cs  (per-partition cs bias,
                    # on ScalarE: PSUM-source is its fast path + frees DVE)
                    nc.scalar.activation(
                        out=kvm[:, 2 * hh + mb, :],
                        in_=ps[:, mb, :],
                        func=AF.Identity,
                        bias=csp_t[:, n0 + mb:n0 + mb + 1],
                        scale=alpha,
                    )
                for ib in range(LB):
                    psv = dv_ps.tile([128, 256], BF16, tag="dv")
                    for db in range(2):
                        nc.tensor.transpose(
                            psv[:, db * 128:(db + 1) * 128],
                            VT_t[:, n0 + db, ib * 128:(ib + 1) * 128],
                            ident,
                        )
                    nc.scalar.copy(
                        out=V_t[:, ib, h * DH:h * DH + 256], in_=psv[:, :]
                    )
            # q_mod = QT * kvm ; pq = phi(q_mod)
            qsl = slice(4 * g, 4 * g + 4)
            nc.vector.tensor_mul(kvm, kvm, QT_t[:, qsl, :])
            eq = hgrp.tile([128, 4, L], BF16, tag="eq")
            nc.vector.tensor_scalar_min(eq, kvm, 0.0)
            nc.scalar.activation(eq, eq, AF.Exp)
            nc.vector.tensor_scalar_max(kvm, kvm, 0.0)
            nc.vector.tensor_add(pq_t[:, qsl, :], eq, kvm)

        # Trigger the sqrt ACT-table load now, off the LN tail's critical path
        # (exp and sqrt live in different table sets; the switch costs ~2.6us).
        warm_sqrt = singles.tile([128, 1], F32)
        nc.scalar.activation(warm_sqrt, eps_t, AF.Sqrt)

        # ---- per-head: A matmul, mask, den, num, outT ----
        attnT_t = big.tile([128, NB, L], BF16, tag="attnT")
        for h in range(H):
            n0 = 2 * h
            # causal block structure of AT[i, l] (i<=l kept):
            #   ib=0: l<128 lower-triangular, l>=128 all-ones
            #   ib=1: l<128 all-zero (skipped entirely), l>=128 triangular
            a_ps = an_ps.tile([128, 2, L], F32, tag="an")
            for db in range(2):
                nc.tensor.matmul(
                    a_ps[:, 0, :],
                    pk_t[:, n0 + db, 0:128],
                    pq_t[:, n0 + db, :],
                    start=(db == 0), stop=(db == 1),
                )
            for db in range(2):
                nc.tensor.matmul(
                    a_ps[:, 1, 128:L],
                    pk_t[:, n0 + db, 128:L],
                    pq_t[:, n0 + db, 128:L],
                    start=(db == 0), stop=(db == 1),
                )
            am = small.tile([128, LB, L], BF16, tag="am")
            nc.vector.tensor_mul(am[:, 0, 0:128], a_ps[:, 0, 0:128],
                                 maskT_t[:, 0, 0:128])
            nc.scalar.copy(out=am[:, 0, 128:L], in_=a_ps[:, 0, 128:L])
            nc.vector.tensor_mul(am[:, 1, 128:L], a_ps[:, 1, 128:L],
                                 maskT_t[:, 1, 128:L])

            den_ps = dv_ps.tile([128, L], F32, tag="dv")
            nc.tensor.matmul(den_ps[:, 0:128], ones_t, am[:, 0, 0:128],
                             start=True, stop=True)
            nc.tensor.matmul(den_ps[:, 128:L], ones_t, am[:, 0, 128:L],
                             start=True, stop=False)
            nc.tensor.matmul(den_ps[:, 128:L], ones_t, am[:, 1, 128:L],
                             start=False, stop=True)
            rden = small.tile([128, L], F32, tag="rden")
            nc.vector.tensor_scalar_max(rden, den_ps, 1e-8)
            nc.vector.reciprocal_approx_fast(out=rden, in_=rden)

            n_ps = an_ps.tile([128, 2, L], F32, tag="an")
            for db in range(2):
                v0 = V_t[:, 0, h * DH + db * 128:h * DH + (db + 1) * 128]
                v1 = V_t[:, 1, h * DH + db * 128:h * DH + (db + 1) * 128]
                nc.tensor.matmul(n_ps[:, db, 0:128], v0, am[:, 0, 0:128],
                                 start=True, stop=True)
                nc.tensor.matmul(n_ps[:, db, 128:L], v0, am[:, 0, 128:L],
                                 start=True, stop=False)
                nc.tensor.matmul(n_ps[:, db, 128:L], v1, am[:, 1, 128:L],
                                 start=False, stop=True)
            for db in range(2):
                nc.vector.tensor_mul(attnT_t[:, n0 + db, :], n_ps[:, db, :], rden)

        # ---- output projection + residual + LayerNorm ----
        x_sb = [big.tile([128, D], F32, tag=f"x{lb}", name=f"x{lb}")
                for lb in range(LB)]
        stats = [small.tile([128, 8, 6], F32, tag=f"stats{lb}",
                            name=f"stats{lb}", bufs=1) for lb in range(LB)]
        for iq in range(8):
            wo_t = prefetched.pop(("o", iq), None)
            if wo_t is None:
                wo_t = panel_dma("o", iq, "w")
            for lb in range(LB):
                ps = proj_ps.tile([128, 2, L], F32, tag="proj")
                for j in range(NB):
                    nc.tensor.matmul(
                        ps[:, 0, :],
                        attnT_t[:, j, lb * 128:(lb + 1) * 128],
                        wo_t[:, j, :],
                        start=(j == 0),
                        stop=(j == NB - 1),
                    )
                # x = o + (query + bo)
                nc.vector.tensor_add(
                    x_sb[lb][:, iq * 256:(iq + 1) * 256],
                    ps[:, 0, :],
                    qres_t[lb][:, iq * 256:(iq + 1) * 256],
                )
                # LN stats pipelined per 256-chunk while later iqs project
                nc.vector.bn_stats(
                    out=stats[lb][:, iq, :],
                    in_=x_sb[lb][:, iq * 256:(iq + 1) * 256],
                )

        for lb in range(LB):
            x = x_sb[lb]
            mv = small.tile([128, 2], F32, tag="mv")
            nc.vector.bn_aggr(out=mv, in_=stats[lb])
            sd = small.tile([128, 1], F32, tag="sd")
            nc.scalar.activation(sd, mv[:, 1:2], AF.Sqrt, bias=eps_t)
            nc.vector.reciprocal_approx_fast(out=sd, in_=sd)
            for ch in range(4):  # quarters, so DVE work overlaps output DMA
                sl = slice(ch * (D // 4), (ch + 1) * (D // 4))
                if plain_ln:
                    # ln_g == 1, ln_b == 0: one fused (x - mu) * rstd
                    nc.vector.tensor_scalar(
                        out=x[:, sl], in0=x[:, sl], scalar1=mv[:, 0:1],
                        scalar2=sd, op0=OP.subtract, op1=OP.mult,
                    )
                else:
                    nc.vector.tensor_scalar(
                        out=x[:, sl], in0=x[:, sl], scalar1=mv[:, 0:1],
                        scalar2=None, op0=OP.subtract,
                    )
                    nc.vector.scalar_tensor_tensor(
                        out=x[:, sl], in0=x[:, sl], scalar=sd, in1=lng_t[:, sl],
                        op0=OP.mult, op1=OP.mult,
                    )
                    nc.vector.tensor_add(x[:, sl], x[:, sl], lnb_t[:, sl])
                nc.sync.dma_start(
                    out=out_d.ap()[lb * 128:(lb + 1) * 128, sl], in_=x[:, sl])


def _host_prep(query, key, value, Wq, Wk, Wv, Wo, bo, ln_g, ln_b, alpha, beta):
    """Host-side: cumulative_state shortcut + layout/dtype marshaling."""
    a, b = float(alpha), float(beta)
    f64 = np.float64
    # mean over (batch, l) of kv[b,h,l,m] = (1/(B*L)) sum_b Ksum[b,h,:].V[b,h,m,:]
    keysum = key.astype(f64).sum(axis=1)                      # [B, D]
    Ksum = (keysum @ Wk.T.astype(f64)).reshape(B, H, DH)      # [B, H, DH]
    WvH = Wv.astype(f64).reshape(H, DH, D)
    wv_eff = np.einsum("hdj,bhd->bhj", WvH, Ksum, optimize=True)      # [B,H,D]
    contrib = np.einsum("bmj,bhj->hm", value.astype(f64), wv_eff, optimize=True)
    mean_kv = contrib / (B * L)                               # [H, DH]
    cs = np.zeros((H, DH), f64)
    c = np.zeros(DH, f64)
    for h in range(H):
        cs[h] = c
        c = b * c + a * mean_kv[h]
    csp = ((1.0 - a) * cs).astype(np.float32)
    csp_dev = np.ascontiguousarray(
        csp.reshape(H, 2, 128).transpose(2, 0, 1).reshape(128, H * 2)
    )
    plain_ln = bool(np.all(ln_g == 1.0) and np.all(ln_b == 0.0))

    bf = ml_dtypes.bfloat16
    qT = np.ascontiguousarray(query.transpose(0, 2, 1)).astype(bf)
    kT = np.ascontiguousarray(key.transpose(0, 2, 1)).astype(bf)
    vT = np.ascontiguousarray(value.transpose(0, 2, 1)).astype(bf)
    wqT = np.ascontiguousarray(Wq.T).astype(bf)
    wkT = np.ascontiguousarray(Wk.T).astype(bf)
    wvT = np.ascontiguousarray(Wv.T).astype(bf)
    woT = np.ascontiguousarray(Wo.T).astype(bf)
    qres = (query + bo[None, None, :]).astype(np.float32)
    maskT = np.triu(np.ones((L, L), np.float32)).astype(bf)   # maskT[i,l]=1 iff i<=l

    in_maps = []
    for c_ in range(B):
        in_maps.append({
            "qT": qT[c_], "kT": kT[c_], "vT": vT[c_],
            "qres": qres[c_],
            "wqT": wqT, "wkT": wkT, "wvT": wvT, "woT": woT,
            "csp": csp_dev, "maskT": maskT,
            "lng": ln_g.astype(np.float32), "lnb": ln_b.astype(np.float32),
        })
    return in_maps, a, plain_ln


def get_nc(alpha: float, plain_ln: bool = True):
    key = (round(float(alpha), 9), bool(plain_ln))
    if key not in _cache:
        _cache[key] = _build(float(alpha), bool(plain_ln))
    return _cache[key]


def kernel(query, key, value, Wq, Wk, Wv, Wo, bo, ln_g, ln_b, alpha, beta,
           _trace=False, _trace_kwargs=None):
    args = [np.asarray(x) for x in
            (query, key, value, Wq, Wk, Wv, Wo, bo, ln_g, ln_b, alpha, beta)]
    in_maps, a, plain_ln = _host_prep(*args)
    nc = get_nc(a, plain_ln)
    res = run_bass_kernel_spmd(
        nc, in_maps, core_ids=list(range(B)),
        trace=_trace, **(_trace_kwargs or {}),
    )
    out = np.stack([res.results[c]["out"] for c in range(B)], axis=0)
    if _trace:
        kernel._last_results = res
    return out


# revision 32
# speedup vs baseline: 1.0395x; 1.0395x over previous
"""DeltaNet attention TRN2 kernel (nn_DeltaNetAttention_5299989643476).

Strategy: data-parallel over batch (8 batches -> 8 NeuronCores). The
cross-batch cumulative_state scan is tiny ([H, Dh]) and is computed on the
host via an algebraic shortcut (mean over (b,l) of kv == Ksum . V
contraction), then passed to every core as a small constant tensor, so the
device program needs no collectives.

On-device, everything runs in a "transposed" layout (features on
partitions, sequence on the free dim):
  - QT/KT/VT projections: weight-stationary bf16 matmuls, fp32 PSUM accum
  - per head: kvT matmul; q-mod via tensor_scalar with per-partition cs;
    phi(x)=elu(x)+1 = relu(x)+exp(min(x,0)); causal linear attention as a
    masked A=pq@pk^T matmul; den via an all-ones stationary matmul (which
    also replicates den across partitions for the division broadcast);
    num needs V back in sequence-major layout -> PE transpose
  - output projection + residual + LayerNorm (bn_stats/bn_aggr)
"""

import numpy as np
import ml_dtypes

import concourse.bass as bass
import concourse.mybir as mybir
import concourse.tile as tile
from concourse import bacc
from concourse.bass_utils import run_bass_kernel_spmd
from concourse.masks import make_identity

B, L, D, H = 8, 256, 2048, 8
DH = D // H            # 256
NB = D // 128          # 16 feature blocks of 128
LB = L // 128          # 2 sequence blocks of 128
EPS = 1e-5

F32 = mybir.dt.float32
BF16 = mybir.dt.bfloat16
AF = mybir.ActivationFunctionType
OP = mybir.AluOpType

_cache = {}


def _build(alpha: float, plain_ln: bool = False):
    nc = bacc.Bacc(
        "TRN2",
        target_bir_lowering=False,
        debug=False,
        enable_asserts=False,
        num_devices=B,
    )

    qT_d = nc.dram_tensor("qT", [D, L], BF16, kind="ExternalInput")
    kT_d = nc.dram_tensor("kT", [D, L], BF16, kind="ExternalInput")
    vT_d = nc.dram_tensor("vT", [D, L], BF16, kind="ExternalInput")
    qres_d = nc.dram_tensor("qres", [L, D], F32, kind="ExternalInput")
    wqT_d = nc.dram_tensor("wqT", [D, D], BF16, kind="ExternalInput")
    wkT_d = nc.dram_tensor("wkT", [D, D], BF16, kind="ExternalInput")
    wvT_d = nc.dram_tensor("wvT", [D, D], BF16, kind="ExternalInput")
    woT_d = nc.dram_tensor("woT", [D, D], BF16, kind="ExternalInput")
    csp_d = nc.dram_tensor("csp", [128, H * 2], F32, kind="ExternalInput")
    maskT_d = nc.dram_tensor("maskT", [L, L], BF16, kind="ExternalInput")
    lng_d = nc.dram_tensor("lng", [D], F32, kind="ExternalInput")
    lnb_d = nc.dram_tensor("lnb", [D], F32, kind="ExternalInput")
    out_d = nc.dram_tensor("out", [L, D], F32, kind="ExternalOutput")

    with tile.TileContext(nc) as tc:
        _body(
            tc, alpha,
            qT_d, kT_d, vT_d, qres_d,
            wqT_d, wkT_d, wvT_d, woT_d,
            csp_d, maskT_d, lng_d, lnb_d, out_d,
            plain_ln,
        )
    nc.compile()
    return nc


def _body(tc, alpha, qT_d, kT_d, vT_d, qres_d, wqT_d, wkT_d, wvT_d, woT_d,
          csp_d, maskT_d, lng_d, lnb_d, out_d, plain_ln):
    nc = tc.nc

    with (
        tc.tile_pool(name="singles", bufs=1) as singles,
        tc.tile_pool(name="wpool", bufs=4) as wpool,
        tc.tile_pool(name="big", bufs=1) as big,
        tc.tile_pool(name="hgrp", bufs=2) as hgrp,
        tc.tile_pool(name="small", bufs=3) as small,
        tc.tile_pool(name="proj_ps", bufs=2, space="PSUM") as proj_ps,
        tc.tile_pool(name="kv_ps", bufs=2, space="PSUM") as kv_ps,
        tc.tile_pool(name="an_ps", bufs=2, space="PSUM") as an_ps,
        tc.tile_pool(name="dv_ps", bufs=2, space="PSUM") as dv_ps,
    ):
        # ---- projections: XT[i, l] = sum_j WT[j, i] * xT[j, l] ----
        # K first (pk depends only on K), then V (kv + transposes), then Q.
        # Inputs stream on the gpsimd queue, weights on the sync queue, so
        # their issue costs overlap. The K input DMA goes first on gpsimd.
        xT_in = {}
        for name, dram in (("k", kT_d), ("v", vT_d), ("q", qT_d)):
            t = big.tile([128, NB, L], BF16, tag=f"{name}T_in", name=f"{name}T_in")
            xT_in[name] = (t, dram)

        def load_xT(name):
            t, dram = xT_in[name]
            r = dram.rearrange("(n p) l -> p n l", p=128)
            # halves so the first j-blocks unblock matmuls sooner
            nc.gpsimd.dma_start(out=t[:, 0:8, :], in_=r[:, 0:8, :])
            nc.gpsimd.dma_start(out=t[:, 8:16, :], in_=r[:, 8:16, :])

        load_xT("k")

        # constants after the K input on the gpsimd queue
        ident = singles.tile([128, 128], BF16)
        make_identity(nc, ident)
        ones_t = singles.tile([128, 128], BF16)
        nc.vector.memset(ones_t, 1.0)
        eps_t = singles.tile([128, 1], F32)
        nc.vector.memset(eps_t, EPS)
        csp_t = singles.tile([128, H * 2], F32)
        nc.gpsimd.dma_start(out=csp_t, in_=csp_d.ap())

        w_rs = {
            "k": wkT_d.rearrange("(n p) i -> p n i", p=128),
            "v": wvT_d.rearrange("(n p) i -> p n i", p=128),
            "q": wqT_d.rearrange("(n p) i -> p n i", p=128),
            "o": woT_d.rearrange("(n p) i -> p n i", p=128),
        }
        succ = {"k": "v", "v": "q", "q": "o"}
        prefetched = {}

        def panel_dma(name, iq, tag, halved=False):
            w_t = wpool.tile([128, NB, 256], BF16, tag=tag, name=f"w_{name}{iq}",
                             bufs=(2 if tag == "wf" else None))
            wsl = slice(iq * 256, (iq + 1) * 256)
            w_r = w_rs[name]
            if halved:
                nc.sync.dma_start(out=w_t[:, 0:8, :], in_=w_r[:, 0:8, wsl])
                nc.sync.dma_start(out=w_t[:, 8:16, :], in_=w_r[:, 8:16, wsl])
            else:
                nc.sync.dma_start(out=w_t, in_=w_r[:, :, wsl])
            return w_t

        projs = {}
        for name in ("k", "v", "q"):
            out_t = big.tile([128, NB, L], BF16, tag=f"{name}proj",
                             name=f"{name}proj")
            x_t = xT_in[name][0]
            for iq in range(8):  # i-quarter: 2 output feature blocks
                w_t = prefetched.pop((name, iq), None)
                if w_t is None:
                    w_t = panel_dma(name, iq, "w", halved=(name == "k" and iq < 2))
                if iq == 3 and succ[name] != "o":
                    # next projection's activation streams during this proj
                    load_xT(succ[name])
                if iq == 5:
                    # next matmul stream's first panel, in dedicated slots, so
                    # the projection boundary has no weight-arrival gap (late
                    # enough that it doesn't steal bandwidth from this stream)
                    nxt = succ[name]
                    prefetched[(nxt, 0)] = panel_dma(nxt, 0, "wf")
                ps = proj_ps.tile([128, 2, L], F32, tag="proj")
                for ib in range(2):
                    for j in range(NB):
                        nc.tensor.matmul(
                            ps[:, ib, :],
                            w_t[:, j, ib * 128:(ib + 1) * 128],
                            x_t[:, j, :],
                            start=(j == 0),
                            stop=(j == NB - 1),
                        )
                for ib in range(2):
                    nc.vector.tensor_copy(out_t[:, iq * 2 + ib, :], ps[:, ib, :])
            projs[name] = out_t
        KT_t, VT_t, QT_t = projs["k"], projs["v"], projs["q"]

        maskT_t = singles.tile([128, LB, L], BF16)
        nc.gpsimd.dma_start(out=maskT_t,
                            in_=maskT_d.rearrange("(a p) l -> p a l", p=128))
        qres_t = []
        for lb in range(LB):
            t = big.tile([128, D], F32, tag=f"qres{lb}", name=f"qres{lb}")
            nc.gpsimd.dma_start(out=t, in_=qres_d.ap()[lb * 128:(lb + 1) * 128, :])
            qres_t.append(t)
        lng_t = lnb_t = None
        if not plain_ln:
            lng_t = singles.tile([128, D], F32)
            nc.gpsimd.dma_start(out=lng_t,
                                in_=lng_d.ap().partition_broadcast(128))
            lnb_t = singles.tile([128, D], F32)
            nc.gpsimd.dma_start(out=lnb_t,
                                in_=lnb_d.ap().partition_broadcast(128))

        # ---- pk = phi(KT) over all heads at once ----
        pk_t = big.tile([128, NB, L], BF16, tag="pk")
        ek_t = big.tile([128, NB, L], BF16, tag="ek")
        nc.vector.tensor_scalar_min(ek_t, KT_t, 0.0)
        nc.scalar.activation(ek_t, ek_t, AF.Exp)
        nc.vector.tensor_scalar_max(pk_t, KT_t, 0.0)
        nc.vector.tensor_add(pk_t, pk_t, ek_t)

        # ---- per-head-group (2 heads): kv + V-transpose + q-mod + phi(q) ----
        # V-transposes ride along per group so PE has filler work while the
        # group's phi chain runs on DVE/ACT.
        V_t = big.tile([128, LB, D], BF16, tag="V")
        pq_t = big.tile([128, NB, L], BF16, tag="pq")
        for g in range(4):  # groups of 2 heads
            kvm = hgrp.tile([128, 4, L], BF16, tag="kvm")
            for hh in range(2):
                h = 2 * g + hh
                n0 = 2 * h
                ps = kv_ps.tile([128, 2, L], F32, tag="kv")
                for mb in range(2):
                    for db in range(2):
                        nc.tensor.matmul(
                            ps[:, mb, :],
                            VT_t[:, n0 + db, mb * 128:(mb + 1) * 128],
                            KT_t[:, n0 + db, :],
                            start=(db == 0),
                            stop=(db == 1),
                        )
                for mb in range(2):
                    # kvm = alpha*kv + (1-alpha)*cs  (per-partition cs bias,
                    # on ScalarE: PSUM-source is its fast path + frees DVE)
                    nc.scalar.activation(
                        out=kvm[:, 2 * hh + mb, :],
                        in_=ps[:, mb, :],
                        func=AF.Identity,
                        bias=csp_t[:, n0 + mb:n0 + mb + 1],
                        scale=alpha,
                    )
                for ib in range(LB):
                    psv = dv_ps.tile([128, 256], BF16, tag="dv")
                    for db in range(2):
                        nc.tensor.transpose(
                            psv[:, db * 128:(db + 1) * 128],
                            VT_t[:, n0 + db, ib * 128:(ib + 1) * 128],
                            ident,
                        )
                    nc.scalar.copy(
                        out=V_t[:, ib, h * DH:h * DH + 256], in_=psv[:, :]
                    )
            # q_mod = QT * kvm ; pq = phi(q_mod)
            qsl = slice(4 * g, 4 * g + 4)
            nc.vector.tensor_mul(kvm, kvm, QT_t[:, qsl, :])
            eq = hgrp.tile([128, 4, L], BF16, tag="eq")
            nc.vector.tensor_scalar_min(eq, kvm, 0.0)
            nc.scalar.activation(eq, eq, AF.Exp)
            nc.vector.tensor_scalar_max(kvm, kvm, 0.0)
            nc.vector.tensor_add(pq_t[:, qsl, :], eq, kvm)

        # Trigger the sqrt ACT-table load now, off the LN tail's critical path
        # (exp and sqrt live in different table sets; the switch costs ~2.6us).
        warm_sqrt = singles.tile([128, 1], F32)
        nc.scalar.activation(warm_sqrt, eps_t, AF.Sqrt)

        # ---- per-head: A matmul, mask, den, num, outT ----
        attnT_t = big.tile([128, NB, L], BF16, tag="attnT")
        for h in range(H):
            n0 = 2 * h
            # causal block structure of AT[i, l] (i<=l kept):
            #   ib=0: l<128 lower-triangular, l>=128 all-ones
            #   ib=1: l<128 all-zero (skipped entirely), l>=128 triangular
            a_ps = an_ps.tile([128, 2, L], F32, tag="an")
            for db in range(2):
                nc.tensor.matmul(
                    a_ps[:, 0, :],
                    pk_t[:, n0 + db, 0:128],
                    pq_t[:, n0 + db, :],
                    start=(db == 0), stop=(db == 1),
                )
            for db in range(2):
                nc.tensor.matmul(
                    a_ps[:, 1, 128:L],
                    pk_t[:, n0 + db, 128:L],
                    pq_t[:, n0 + db, 128:L],
                    start=(db == 0), stop=(db == 1),
                )
            am = small.tile([128, LB, L], BF16, tag="am")
            nc.vector.tensor_mul(am[:, 0, 0:128], a_ps[:, 0, 0:128],
                                 maskT_t[:, 0, 0:128])
            nc.scalar.copy(out=am[:, 0, 128:L], in_=a_ps[:, 0, 128:L])
            nc.vector.tensor_mul(am[:, 1, 128:L], a_ps[:, 1, 128:L],
                                 maskT_t[:, 1, 128:L])

            den_ps = dv_ps.tile([128, L], F32, tag="dv")
            nc.tensor.matmul(den_ps[:, 0:128], ones_t, am[:, 0, 0:128],
                             start=True, stop=True)
            nc.tensor.matmul(den_ps[:, 128:L], ones_t, am[:, 0, 128:L],
                             start=True, stop=False)
            nc.tensor.matmul(den_ps[:, 128:L], ones_t, am[:, 1, 128:L],
                             start=False, stop=True)
            rden = small.tile([128, L], F32, tag="rden")
            nc.vector.tensor_scalar_max(rden, den_ps, 1e-8)
            nc.vector.reciprocal_approx_fast(out=rden, in_=rden)

            n_ps = an_ps.tile([128, 2, L], F32, tag="an")
            for db in range(2):
                v0 = V_t[:, 0, h * DH + db * 128:h * DH + (db + 1) * 128]
                v1 = V_t[:, 1, h * DH + db * 128:h * DH + (db + 1) * 128]
                nc.tensor.matmul(n_ps[:, db, 0:128], v0, am[:, 0, 0:128],
                                 start=True, stop=True)
                nc.tensor.matmul(n_ps[:, db, 128:L], v0, am[:, 0, 128:L],
                                 start=True, stop=False)
                nc.tensor.matmul(n_ps[:, db, 128:L], v1, am[:, 1, 128:L],
                                 start=False, stop=True)
            for db in range(2):
                nc.vector.tensor_mul(attnT_t[:, n0 + db, :], n_ps[:, db, :], rden)

        # ---- output projection + residual + LayerNorm ----
        x_sb = [big.tile([128, D], F32, tag=f"x{lb}", name=f"x{lb}")
                for lb in range(LB)]
        stats = [small.tile([128, 8, 6], F32, tag=f"stats{lb}",
                            name=f"stats{lb}", bufs=1) for lb in range(LB)]
        for iq in range(8):
            wo_t = prefetched.pop(("o", iq), None)
            if wo_t is None:
                wo_t = panel_dma("o", iq, "w")
            for lb in range(LB):
                ps = proj_ps.tile([128, 2, L], F32, tag="proj")
                for j in range(NB):
                    nc.tensor.matmul(
                        ps[:, 0, :],
                        attnT_t[:, j, lb * 128:(lb + 1) * 128],
                        wo_t[:, j, :],
                        start=(j == 0),
                        stop=(j == NB - 1),
                    )
                # x = o + (query + bo)
                nc.vector.tensor_add(
                    x_sb[lb][:, iq * 256:(iq + 1) * 256],
                    ps[:, 0, :],
                    qres_t[lb][:, iq * 256:(iq + 1) * 256],
                )
                # LN stats pipelined per 256-chunk while later iqs project
                nc.vector.bn_stats(
                    out=stats[lb][:, iq, :],
                    in_=x_sb[lb][:, iq * 256:(iq + 1) * 256],
                )

        for lb in range(LB):
            x = x_sb[lb]
            mv = small.tile([128, 2], F32, tag="mv")
            nc.vector.bn_aggr(out=mv, in_=stats[lb])
            sd = small.tile([128, 1], F32, tag="sd")
            nc.scalar.activation(sd, mv[:, 1:2], AF.Sqrt, bias=eps_t)
            nc.vector.reciprocal_approx_fast(out=sd, in_=sd)
            for ch in range(4):  # quarters, so DVE work overlaps output DMA
                sl = slice(ch * (D // 4), (ch + 1) * (D // 4))
                if plain_ln:
                    # ln_g == 1, ln_b == 0: one fused (x - mu) * rstd
                    nc.vector.tensor_scalar(
                        out=x[:, sl], in0=x[:, sl], scalar1=mv[:, 0:1],
                        scalar2=sd, op0=OP.subtract, op1=OP.mult,
                    )
                else:
                    nc.vector.tensor_scalar(
                        out=x[:, sl], in0=x[:, sl], scalar1=mv[:, 0:1],
                        scalar2=None, op0=OP.subtract,
                    )
                    nc.vector.scalar_tensor_tensor(
                        out=x[:, sl], in0=x[:, sl], scalar=sd, in1=lng_t[:, sl],
                        op0=OP.mult, op1=OP.mult,
                    )
                    nc.vector.tensor_add(x[:, sl], x[:, sl], lnb_t[:, sl])
                nc.sync.dma_start(
                    out=out_d.ap()[lb * 128:(lb + 1) * 128, sl], in_=x[:, sl])


def _host_prep(query, key, value, Wq, Wk, Wv, Wo, bo, ln_g, ln_b, alpha, beta):
    """Host-side: cumulative_state shortcut + layout/dtype marshaling."""
    a, b = float(alpha), float(beta)
    f64 = np.float64
    # mean over (batch, l) of kv[b,h,l,m] = (1/(B*L)) sum_b Ksum[b,h,:].V[b,h,m,:]
    keysum = key.astype(f64).sum(axis=1)                      # [B, D]
    Ksum = (keysum @ Wk.T.astype(f64)).reshape(B, H, DH)      # [B, H, DH]
    WvH = Wv.astype(f64).reshape(H, DH, D)
    wv_eff = np.einsum("hdj,bhd->bhj", WvH, Ksum, optimize=True)      # [B,H,D]
    contrib = np.einsum("bmj,bhj->hm", value.astype(f64), wv_eff, optimize=True)
    mean_kv = contrib / (B * L)                               # [H, DH]
    cs = np.zeros((H, DH), f64)
    c = np.zeros(DH, f64)
    for h in range(H):
        cs[h] = c
        c = b * c + a * mean_kv[h]
    csp = ((1.0 - a) * cs).astype(np.float32)
    csp_dev = np.ascontiguousarray(
        csp.reshape(H, 2, 128).transpose(2, 0, 1).reshape(128, H * 2)
    )
    plain_ln = bool(np.all(ln_g == 1.0) and np.all(ln_b == 0.0))

    bf = ml_dtypes.bfloat16
    qT = np.ascontiguousarray(query.transpose(0, 2, 1)).astype(bf)
    kT = np.ascontiguousarray(key.transpose(0, 2, 1)).astype(bf)
    vT = np.ascontiguousarray(value.transpose(0, 2, 1)).astype(bf)
    wqT = np.ascontiguousarray(Wq.T).astype(bf)
    wkT = np.ascontiguousarray(Wk.T).astype(bf)
    wvT = np.ascontiguousarray(Wv.T).astype(bf)
    woT = np.ascontiguousarray(Wo.T).astype(bf)
    qres = (query + bo[None, None, :]).astype(np.float32)
    maskT = np.triu(np.ones((L, L), np.float32)).astype(bf)   # maskT[i,l]=1 iff i<=l

    in_maps = []
    for c_ in range(B):
        in_maps.append({
            "qT": qT[c_], "kT": kT[c_], "vT": vT[c_],
            "qres": qres[c_],
            "wqT": wqT, "wkT": wkT, "wvT": wvT, "woT": woT,
            "csp": csp_dev, "maskT": maskT,
            "lng": ln_g.astype(np.float32), "lnb": ln_b.astype(np.float32),
        })
    return in_maps, a, plain_ln


def get_nc(alpha: float, plain_ln: bool = True):
    key = (round(float(alpha), 9), bool(plain_ln))
    if key not in _cache:
        _cache[key] = _build(float(alpha), bool(plain_ln))
    return _cache[key]


def kernel(query, key, value, Wq, Wk, Wv, Wo, bo, ln_g, ln_b, alpha, beta,
           _trace=False, _trace_kwargs=None):
    args = [np.asarray(x) for x in
            (query, key, value, Wq, Wk, Wv, Wo, bo, ln_g, ln_b, alpha, beta)]
    in_maps, a, plain_ln = _host_prep(*args)
    nc = get_nc(a, plain_ln)
    res = run_bass_kernel_spmd(
        nc, in_maps, core_ids=list(range(B)),
        trace=_trace, **(_trace_kwargs or {}),
    )
    out = np.stack([res.results[c]["out"] for c in range(B)], axis=0)
    if _trace:
        kernel._last_results = res
    return out


# revision 39
# speedup vs baseline: 1.0833x; 1.0422x over previous
"""DeltaNet attention TRN2 kernel (nn_DeltaNetAttention_5299989643476).

Strategy: data-parallel over batch (8 batches -> 8 NeuronCores). The
cross-batch cumulative_state scan is tiny ([H, Dh]) and is computed on the
host via an algebraic shortcut (mean over (b,l) of kv == Ksum . V
contraction), then passed to every core as a small constant tensor, so the
device program needs no collectives.

On-device, everything runs in a "transposed" layout (features on
partitions, sequence on the free dim):
  - QT/KT/VT projections: weight-stationary bf16 matmuls, fp32 PSUM accum
  - per head: kvT matmul; q-mod via tensor_scalar with per-partition cs;
    phi(x)=elu(x)+1 = relu(x)+exp(min(x,0)); causal linear attention as a
    masked A=pq@pk^T matmul; den via an all-ones stationary matmul (which
    also replicates den across partitions for the division broadcast);
    num needs V back in sequence-major layout -> PE transpose
  - output projection + residual + LayerNorm (bn_stats/bn_aggr)
"""

import numpy as np
import ml_dtypes

import concourse.bass as bass
import concourse.mybir as mybir
import concourse.tile as tile
from concourse import bacc
from concourse.bass_utils import run_bass_kernel_spmd
from concourse.masks import make_identity

B, L, D, H = 8, 256, 2048, 8
DH = D // H            # 256
NB = D // 128          # 16 feature blocks of 128
LB = L // 128          # 2 sequence blocks of 128
EPS = 1e-5

F32 = mybir.dt.float32
BF16 = mybir.dt.bfloat16
AF = mybir.ActivationFunctionType
OP = mybir.AluOpType

_cache = {}


def _build(alpha: float, plain_ln: bool = False):
    nc = bacc.Bacc(
        "TRN2",
        target_bir_lowering=False,
        debug=False,
        enable_asserts=False,
        num_devices=B,
    )

    qT_d = nc.dram_tensor("qT", [D, L], BF16, kind="ExternalInput")
    kT_d = nc.dram_tensor("kT", [D, L], BF16, kind="ExternalInput")
    vT_d = nc.dram_tensor("vT", [D, L], BF16, kind="ExternalInput")
    qres_d = nc.dram_tensor("qres", [L, D], F32, kind="ExternalInput")
    wqT_d = nc.dram_tensor("wqT", [D, D], BF16, kind="ExternalInput")
    wkT_d = nc.dram_tensor("wkT", [D, D], BF16, kind="ExternalInput")
    wvT_d = nc.dram_tensor("wvT", [D, D], BF16, kind="ExternalInput")
    woT_d = nc.dram_tensor("woT", [D, D], BF16, kind="ExternalInput")
    csp_d = nc.dram_tensor("csp", [128, H * 2], F32, kind="ExternalInput")
    maskT_d = nc.dram_tensor("maskT", [L, L], BF16, kind="ExternalInput")
    lng_d = nc.dram_tensor("lng", [D], F32, kind="ExternalInput")
    lnb_d = nc.dram_tensor("lnb", [D], F32, kind="ExternalInput")
    out_d = nc.dram_tensor("out", [L, D], F32, kind="ExternalOutput")

    with tile.TileContext(nc) as tc:
        _body(
            tc, alpha,
            qT_d, kT_d, vT_d, qres_d,
            wqT_d, wkT_d, wvT_d, woT_d,
            csp_d, maskT_d, lng_d, lnb_d, out_d,
            plain_ln,
        )
    nc.compile()
    return nc


def _body(tc, alpha, qT_d, kT_d, vT_d, qres_d, wqT_d, wkT_d, wvT_d, woT_d,
          csp_d, maskT_d, lng_d, lnb_d, out_d, plain_ln):
    nc = tc.nc

    with (
        tc.tile_pool(name="singles", bufs=1) as singles,
        tc.tile_pool(name="wpool", bufs=6) as wpool,
        tc.tile_pool(name="big", bufs=1) as big,
        tc.tile_pool(name="hgrp", bufs=2) as hgrp,
        tc.tile_pool(name="small", bufs=3) as small,
        tc.tile_pool(name="proj_ps", bufs=2, space="PSUM") as proj_ps,
        tc.tile_pool(name="kv_ps", bufs=2, space="PSUM") as kv_ps,
        tc.tile_pool(name="an_ps", bufs=2, space="PSUM") as an_ps,
        tc.tile_pool(name="dv_ps", bufs=2, space="PSUM") as dv_ps,
    ):
        # ---- projections: XT[i, l] = sum_j WT[j, i] * xT[j, l] ----
        # K first (pk depends only on K), then V (kv + transposes), then Q.
        # Inputs stream on the gpsimd queue, weights on the sync queue, so
        # their issue costs overlap. The K input DMA goes first on gpsimd.
        xT_in = {}
        for name, dram in (("k", kT_d), ("v", vT_d), ("q", qT_d)):
            t = big.tile([128, NB, L], BF16, tag=f"{name}T_in", name=f"{name}T_in")
            xT_in[name] = (t, dram)

        def load_xT(name):
            t, dram = xT_in[name]
            r = dram.rearrange("(n p) l -> p n l", p=128)
            # halves so the first j-blocks unblock matmuls sooner
            nc.gpsimd.dma_start(out=t[:, 0:8, :], in_=r[:, 0:8, :])
            nc.gpsimd.dma_start(out=t[:, 8:16, :], in_=r[:, 8:16, :])

        load_xT("k")

        # constants after the K input on the gpsimd queue
        ident = singles.tile([128, 128], BF16)
        make_identity(nc, ident)
        ones_t = singles.tile([128, 128], BF16)
        nc.vector.memset(ones_t, 1.0)
        eps_t = singles.tile([128, 1], F32)
        nc.vector.memset(eps_t, EPS)
        csp_t = singles.tile([128, H * 2], F32)
        nc.gpsimd.dma_start(out=csp_t, in_=csp_d.ap())

        w_rs = {
            "k": wkT_d.rearrange("(n p) i -> p n i", p=128),
            "v": wvT_d.rearrange("(n p) i -> p n i", p=128),
            "q": wqT_d.rearrange("(n p) i -> p n i", p=128),
            "o": woT_d.rearrange("(n p) i -> p n i", p=128),
        }
        succ = {"k": "v", "v": "q", "q": "o"}
        prefetched = {}

        def panel_dma(name, iq, tag, halved=False):
            w_t = wpool.tile([128, NB, 256], BF16, tag=tag, name=f"w_{name}{iq}")
            wsl = slice(iq * 256, (iq + 1) * 256)
            w_r = w_rs[name]
            if halved:
                nc.sync.dma_start(out=w_t[:, 0:8, :], in_=w_r[:, 0:8, wsl])
                nc.sync.dma_start(out=w_t[:, 8:16, :], in_=w_r[:, 8:16, wsl])
            else:
                nc.sync.dma_start(out=w_t, in_=w_r[:, :, wsl])
            return w_t

        projs = {}
        for name in ("k", "v", "q"):
            out_t = big.tile([128, NB, L], BF16, tag=f"{name}proj",
                             name=f"{name}proj")
            x_t = xT_in[name][0]
            for iq in range(8):  # i-quarter: 2 output feature blocks
                w_t = prefetched.pop((name, iq), None)
                if w_t is None:
                    w_t = panel_dma(name, iq, "w", halved=(name == "k" and iq < 2))
                if iq == 3 and succ[name] != "o":
                    # next projection's activation streams during this proj
                    load_xT(succ[name])
                ps = proj_ps.tile([128, 2, L], F32, tag="proj")
                for ib in range(2):
                    for j in range(NB):
                        nc.tensor.matmul(
                            ps[:, ib, :],
                            w_t[:, j, ib * 128:(ib + 1) * 128],
                            x_t[:, j, :],
                            start=(j == 0),
                            stop=(j == NB - 1),
                        )
                for ib in range(2):
                    nc.vector.tensor_copy(out_t[:, iq * 2 + ib, :], ps[:, ib, :])
            projs[name] = out_t
        KT_t, VT_t, QT_t = projs["k"], projs["v"], projs["q"]

        maskT_t = singles.tile([128, LB, L], BF16)
        nc.gpsimd.dma_start(out=maskT_t,
                            in_=maskT_d.rearrange("(a p) l -> p a l", p=128))
        qres_t = []
        for lb in range(LB):
            t = big.tile([128, D], F32, tag=f"qres{lb}", name=f"qres{lb}")
            nc.gpsimd.dma_start(out=t, in_=qres_d.ap()[lb * 128:(lb + 1) * 128, :])
            qres_t.append(t)
        lng_t = lnb_t = None
        if not plain_ln:
            lng_t = singles.tile([128, D], F32)
            nc.gpsimd.dma_start(out=lng_t,
                                in_=lng_d.ap().partition_broadcast(128))
            lnb_t = singles.tile([128, D], F32)
            nc.gpsimd.dma_start(out=lnb_t,
                                in_=lnb_d.ap().partition_broadcast(128))

        # ---- pk = phi(KT) over all heads at once ----
        pk_t = big.tile([128, NB, L], BF16, tag="pk")
        ek_t = big.tile([128, NB, L], BF16, tag="ek")
        nc.vector.tensor_scalar_min(ek_t, KT_t, 0.0)
        nc.scalar.activation(ek_t, ek_t, AF.Exp)
        nc.vector.tensor_scalar_max(pk_t, KT_t, 0.0)
        nc.vector.tensor_add(pk_t, pk_t, ek_t)

        # ---- per-head-group (2 heads): kv + V-transpose + q-mod + phi(q) ----
        # V-transposes ride along per group so PE has filler work while the
        # group's phi chain runs on DVE/ACT.
        V_t = big.tile([128, LB, D], BF16, tag="V")
        pq_t = big.tile([128, NB, L], BF16, tag="pq")
        for g in range(4):  # groups of 2 heads
            kvm = hgrp.tile([128, 4, L], BF16, tag="kvm")
            for hh in range(2):
                h = 2 * g + hh
                n0 = 2 * h
                ps = kv_ps.tile([128, 2, L], F32, tag="kv")
                for mb in range(2):
                    for db in range(2):
                        nc.tensor.matmul(
                            ps[:, mb, :],
                            VT_t[:, n0 + db, mb * 128:(mb + 1) * 128],
                            KT_t[:, n0 + db, :],
                            start=(db == 0),
                            stop=(db == 1),
                        )
                for mb in range(2):
                    # q_mod = (alpha*Q) * (kv + cs*(1-alpha)/alpha); the
                    # alpha factor is folded into Wq on the host, so one STT
                    # straight from PSUM does modulate+multiply.
                    nc.vector.scalar_tensor_tensor(
                        out=kvm[:, 2 * hh + mb, :],
                        in0=ps[:, mb, :],
                        scalar=csp_t[:, n0 + mb:n0 + mb + 1],
                        in1=QT_t[:, n0 + mb, :],
                        op0=OP.add,
                        op1=OP.mult,
                    )
                for ib in range(LB):
                    psv = dv_ps.tile([128, 256], BF16, tag="dv")
                    for db in range(2):
                        nc.tensor.transpose(
                            psv[:, db * 128:(db + 1) * 128],
                            VT_t[:, n0 + db, ib * 128:(ib + 1) * 128],
                            ident,
                        )
                    nc.scalar.copy(
                        out=V_t[:, ib, h * DH:h * DH + 256], in_=psv[:, :]
                    )
            # pq = phi(q_mod)
            qsl = slice(4 * g, 4 * g + 4)
            eq = hgrp.tile([128, 4, L], BF16, tag="eq")
            nc.vector.tensor_scalar_min(eq, kvm, 0.0)
            nc.scalar.activation(eq, eq, AF.Exp)
            nc.vector.tensor_scalar_max(kvm, kvm, 0.0)
            nc.vector.tensor_add(pq_t[:, qsl, :], eq, kvm)

        # Trigger the sqrt ACT-table load now, off the LN tail's critical path
        # (exp and sqrt live in different table sets; the switch costs ~2.6us).
        warm_sqrt = singles.tile([128, 1], F32)
        nc.scalar.activation(warm_sqrt, eps_t, AF.Sqrt)

        # ---- per-head: A matmul, mask, den, num, outT ----
        attnT_t = big.tile([128, NB, L], BF16, tag="attnT")
        for h in range(H):
            n0 = 2 * h
            # causal block structure of AT[i, l] (i<=l kept):
            #   ib=0: l<128 lower-triangular, l>=128 all-ones
            #   ib=1: l<128 all-zero (skipped entirely), l>=128 triangular
            a_ps = an_ps.tile([128, 2, L], F32, tag="an")
            for db in range(2):
                nc.tensor.matmul(
                    a_ps[:, 0, :],
                    pk_t[:, n0 + db, 0:128],
                    pq_t[:, n0 + db, :],
                    start=(db == 0), stop=(db == 1),
                )
            for db in range(2):
                nc.tensor.matmul(
                    a_ps[:, 1, 128:L],
                    pk_t[:, n0 + db, 128:L],
                    pq_t[:, n0 + db, 128:L],
                    start=(db == 0), stop=(db == 1),
                )
            am = small.tile([128, LB, L], BF16, tag="am")
            nc.vector.tensor_mul(am[:, 0, 0:128], a_ps[:, 0, 0:128],
                                 maskT_t[:, 0, 0:128])
            nc.scalar.copy(out=am[:, 0, 128:L], in_=a_ps[:, 0, 128:L])
            nc.vector.tensor_mul(am[:, 1, 128:L], a_ps[:, 1, 128:L],
                                 maskT_t[:, 1, 128:L])

            den_ps = dv_ps.tile([128, L], F32, tag="dv")
            nc.tensor.matmul(den_ps[:, 0:128], ones_t, am[:, 0, 0:128],
                             start=True, stop=True)
            nc.tensor.matmul(den_ps[:, 128:L], ones_t, am[:, 0, 128:L],
                             start=True, stop=False)
            nc.tensor.matmul(den_ps[:, 128:L], ones_t, am[:, 1, 128:L],
                             start=False, stop=True)
            rden = small.tile([128, L], F32, tag="rden")
            nc.vector.tensor_scalar_max(rden, den_ps, 1e-8)
            nc.vector.reciprocal_approx_fast(out=rden, in_=rden)

            n_ps = an_ps.tile([128, 2, L], F32, tag="an")
            for db in range(2):
                v0 = V_t[:, 0, h * DH + db * 128:h * DH + (db + 1) * 128]
                v1 = V_t[:, 1, h * DH + db * 128:h * DH + (db + 1) * 128]
                nc.tensor.matmul(n_ps[:, db, 0:128], v0, am[:, 0, 0:128],
                                 start=True, stop=True)
                nc.tensor.matmul(n_ps[:, db, 128:L], v0, am[:, 0, 128:L],
                                 start=True, stop=False)
                nc.tensor.matmul(n_ps[:, db, 128:L], v1, am[:, 1, 128:L],
                                 start=False, stop=True)
            for db in range(2):
                nc.vector.tensor_mul(attnT_t[:, n0 + db, :], n_ps[:, db, :], rden)

        # ---- output projection + residual + LayerNorm ----
        x_sb = [big.tile([128, D], F32, tag=f"x{lb}", name=f"x{lb}")
                for lb in range(LB)]
        stats = [small.tile([128, 8, 6], F32, tag=f"stats{lb}",
                            name=f"stats{lb}", bufs=1) for lb in range(LB)]
        for iq in range(8):
            wo_t = prefetched.pop(("o", iq), None)
            if wo_t is None:
                wo_t = panel_dma("o", iq, "w")
            for lb in range(LB):
                ps = proj_ps.tile([128, 2, L], F32, tag="proj")
                for j in range(NB):
                    nc.tensor.matmul(
                        ps[:, 0, :],
                        attnT_t[:, j, lb * 128:(lb + 1) * 128],
                        wo_t[:, j, :],
                        start=(j == 0),
                        stop=(j == NB - 1),
                    )
                # x = o + (query + bo)
                nc.vector.tensor_add(
                    x_sb[lb][:, iq * 256:(iq + 1) * 256],
                    ps[:, 0, :],
                    qres_t[lb][:, iq * 256:(iq + 1) * 256],
                )
                # LN stats pipelined per 256-chunk while later iqs project
                nc.vector.bn_stats(
                    out=stats[lb][:, iq, :],
                    in_=x_sb[lb][:, iq * 256:(iq + 1) * 256],
                )

        for lb in range(LB):
            x = x_sb[lb]
            mv = small.tile([128, 2], F32, tag="mv")
            nc.vector.bn_aggr(out=mv, in_=stats[lb])
            sd = small.tile([128, 1], F32, tag="sd")
            nc.scalar.activation(sd, mv[:, 1:2], AF.Sqrt, bias=eps_t)
            nc.vector.reciprocal_approx_fast(out=sd, in_=sd)
            for ch in range(4):  # quarters, so DVE work overlaps output DMA
                sl = slice(ch * (D // 4), (ch + 1) * (D // 4))
                if plain_ln:
                    # ln_g == 1, ln_b == 0: one fused (x - mu) * rstd
                    nc.vector.tensor_scalar(
                        out=x[:, sl], in0=x[:, sl], scalar1=mv[:, 0:1],
                        scalar2=sd, op0=OP.subtract, op1=OP.mult,
                    )
                else:
                    nc.vector.tensor_scalar(
                        out=x[:, sl], in0=x[:, sl], scalar1=mv[:, 0:1],
                        scalar2=None, op0=OP.subtract,
                    )
                    nc.vector.scalar_tensor_tensor(
                        out=x[:, sl], in0=x[:, sl], scalar=sd, in1=lng_t[:, sl],
                        op0=OP.mult, op1=OP.mult,
                    )
                    nc.vector.tensor_add(x[:, sl], x[:, sl], lnb_t[:, sl])
                nc.sync.dma_start(
                    out=out_d.ap()[lb * 128:(lb + 1) * 128, sl], in_=x[:, sl])


def _host_prep(query, key, value, Wq, Wk, Wv, Wo, bo, ln_g, ln_b, alpha, beta):
    """Host-side: cumulative_state shortcut + layout/dtype marshaling."""
    a, b = float(alpha), float(beta)
    f64 = np.float64
    # mean over (batch, l) of kv[b,h,l,m] = (1/(B*L)) sum_b Ksum[b,h,:].V[b,h,m,:]
    keysum = key.astype(f64).sum(axis=1)                      # [B, D]
    Ksum = (keysum @ Wk.T.astype(f64)).reshape(B, H, DH)      # [B, H, DH]
    WvH = Wv.astype(f64).reshape(H, DH, D)
    wv_eff = np.einsum("hdj,bhd->bhj", WvH, Ksum, optimize=True)      # [B,H,D]
    contrib = np.einsum("bmj,bhj->hm", value.astype(f64), wv_eff, optimize=True)
    mean_kv = contrib / (B * L)                               # [H, DH]
    cs = np.zeros((H, DH), f64)
    c = np.zeros(DH, f64)
    for h in range(H):
        cs[h] = c
        c = b * c + a * mean_kv[h]
    # q_mod = Q*((1-a)*cs + a*kv) = (a*Q)*(kv + (1-a)/a*cs); a is folded
    # into Wq below, and this is cs*(1-a)/a:
    csp = ((1.0 - a) / a * cs if a != 0 else 0.0 * cs).astype(np.float32)
    csp_dev = np.ascontiguousarray(
        csp.reshape(H, 2, 128).transpose(2, 0, 1).reshape(128, H * 2)
    )
    plain_ln = bool(np.all(ln_g == 1.0) and np.all(ln_b == 0.0))

    bf = ml_dtypes.bfloat16
    qT = np.ascontiguousarray(query.transpose(0, 2, 1)).astype(bf)
    kT = np.ascontiguousarray(key.transpose(0, 2, 1)).astype(bf)
    vT = np.ascontiguousarray(value.transpose(0, 2, 1)).astype(bf)
    wqT = np.ascontiguousarray(a * Wq.T).astype(bf)
    wkT = np.ascontiguousarray(Wk.T).astype(bf)
    wvT = np.ascontiguousarray(Wv.T).astype(bf)
    woT = np.ascontiguousarray(Wo.T).astype(bf)
    qres = (query + bo[None, None, :]).astype(np.float32)
    maskT = np.triu(np.ones((L, L), np.float32)).astype(bf)   # maskT[i,l]=1 iff i<=l

    in_maps = []
    for c_ in range(B):
        in_maps.append({
            "qT": qT[c_], "kT": kT[c_], "vT": vT[c_],
            "qres": qres[c_],
            "wqT": wqT, "wkT": wkT, "wvT": wvT, "woT": woT,
            "csp": csp_dev, "maskT": maskT,
            "lng": ln_g.astype(np.float32), "lnb": ln_b.astype(np.float32),
        })
    return in_maps, a, plain_ln


def get_nc(alpha: float, plain_ln: bool = True):
    key = (round(float(alpha), 9), bool(plain_ln))
    if key not in _cache:
        _cache[key] = _build(float(alpha), bool(plain_ln))
    return _cache[key]


def kernel(query, key, value, Wq, Wk, Wv, Wo, bo, ln_g, ln_b, alpha, beta,
           _trace=False, _trace_kwargs=None):
    args = [np.asarray(x) for x in
            (query, key, value, Wq, Wk, Wv, Wo, bo, ln_g, ln_b, alpha, beta)]
    in_maps, a, plain_ln = _host_prep(*args)
    nc = get_nc(a, plain_ln)
    res = run_bass_kernel_spmd(
        nc, in_maps, core_ids=list(range(B)),
        trace=_trace, **(_trace_kwargs or {}),
    )
    out = np.stack([res.results[c]["out"] for c in range(B)], axis=0)
    if _trace:
        kernel._last_results = res
    return out
